# revision 1
# baseline (speedup 1.0000x reference)
"""D-MPNN encoder layer on 8 Trainium2 NeuronCores (Bass/Tile).

Sharding strategy
-----------------
Edge pairs are partitioned across 8 cores (50k pairs -> 100k directed edges per
core) and the node space is split into two halves (NH=25088) so every gather
table has < 32768 rows (int16 dma_gather indices).

Per core, edges are grouped into 4 classes by (src-half, dst-half), ordered so
that the reverse edge of class (s,d) slot i is class (d,s) slot i (off-diag)
or slot i +/- F (diagonal): the h[rev] read is a sequential DMA, never a
gather.

Per message layer (fp16 storage, fp32 PSUM accumulation):
  m = dma_gather(tmp_half[src]) - h_prev[rev]        (both edge-major)
  h = relu(W_h @ m) via PE transpose of each 128-edge chunk + stationary
      matmuls -> edge-major f16
Segment-sum: per dst-window (128 nodes), edge rows are re-gathered and reduced
with DVE-generated one-hot selection matmuls accumulating in PSUM.

Collectives are split per node half and fired as soon as that half's seg-sum
is done, so each AllReduce overlaps the other half's seg-sum and the next
layer's first two edge classes (which only need the first half's table).
The initial atom projection is computed distributed (1/8 per core, two 3136-
row chunks, one per node half) and AllGathered per half. The final layer
ReduceScatters each half so core c ends up with rows [c*3136,(c+1)*3136) of
both halves; the host reassembles.
"""

import sys
import numpy as np

sys.path.insert(0, "/opt/trn_rl_repo")

# ---------------------------------------------------------------- constants
N_NODES = 50000
N_PAIRS = 400000
ATOM_FDIM = 133
BOND_FDIM = 14
HIDDEN = 128
DEPTH = 3
N_CORES = 8
NH = 25088                      # node half size (196 windows of 128)
NH8 = NH // N_CORES             # 3136 rows per core per half

GOP = 4096                      # rows per dma_gather instruction
BLK = 512                       # matmul free-dim block
OV = 16                         # overflow slots per (class, window)
CAP = 128 + OV                  # max edges per (core, class, window)

F16 = np.float16
I16 = np.int16

CLS_NAMES = ["00", "01", "10", "11"]


def _derived():
    npad = 2 * NH
    return npad, NH // 128, 2 * NH8


def _wrap_idx(idx):
    """int16 index array -> dma_gather SBUF layout [128, n/16]."""
    n = idx.shape[0]
    assert n % 16 == 0
    return np.tile(idx.reshape(n // 16, 16).T, (8, 1)).copy()


def _ceil_to(x, m):
    return ((x + m - 1) // m) * m


def _balance_pairs(u, v):
    """Assign pairs to cores so no (core, class, dst-window) exceeds CAP.

    Start from round-robin blocks, then move pairs out of overloaded cells.
    Each pair hits two cells: (class(u,v), window(v)) and (class(v,u),
    window(u)).
    """
    NWIN = NH // 128
    hu = (u >= NH).astype(np.int64)
    hv = (v >= NH).astype(np.int64)
    cell1 = (hu * 2 + hv) * NWIN + (v - hv * NH) // 128
    cell2 = (hv * 2 + hu) * NWIN + (u - hu * NH) // 128
    assign = np.repeat(np.arange(N_CORES), N_PAIRS // N_CORES)
    loads = np.zeros((N_CORES, 4 * NWIN), np.int64)
    np.add.at(loads, (assign, cell1), 1)
    np.add.at(loads, (assign, cell2), 1)
    rng = np.random.default_rng(0)
    for _ in range(200):
        over = loads > CAP
        if not over.any():
            break
        bad = np.nonzero(over[assign, cell1] | over[assign, cell2])[0]
        rng.shuffle(bad)
        for p in bad:
            c = assign[p]
            if loads[c, cell1[p]] <= CAP and loads[c, cell2[p]] <= CAP:
                continue
            score = np.maximum(loads[:, cell1[p]], loads[:, cell2[p]])
            score[c] += 10000
            cand = int(np.argmin(score))
            if max(loads[cand, cell1[p]], loads[cand, cell2[p]]) + 1 <= CAP:
                loads[c, cell1[p]] -= 1
                loads[c, cell2[p]] -= 1
                loads[cand, cell1[p]] += 1
                loads[cand, cell2[p]] += 1
                assign[p] = cand
    assert (loads <= CAP).all(), "pair balancing failed"
    return assign


def _host_prep(atom_feats, bond_feats, W_i, W_h, W_o, src, dst):
    NPAD, NWIN, OUT_COLS = _derived()
    src = np.asarray(src).astype(np.int64)
    dst = np.asarray(dst).astype(np.int64)
    u = src[:N_PAIRS]
    v = dst[:N_PAIRS]

    assign = _balance_pairs(u, v)
    per_core = []
    for c in range(N_CORES):
        sel = np.nonzero(assign == c)[0]
        pu = u[sel]
        pv = v[sel]
        gp = sel
        per_core.append((pu, pv, gp, (pu >= NH).astype(np.int8),
                         (pv >= NH).astype(np.int8)))

    n00 = max(int(((p[3] == 0) & (p[4] == 0)).sum()) for p in per_core)
    n11 = max(int(((p[3] == 1) & (p[4] == 1)).sum()) for p in per_core)
    n01 = max(int(((p[3] != p[4])).sum()) for p in per_core)
    F00 = _ceil_to(max(n00, 512), 1024)
    F11 = _ceil_to(max(n11, 512), 1024)
    S01 = _ceil_to(max(n01, 1024), 1024)
    sizes = {"00": 2 * F00, "01": S01, "10": S01, "11": 2 * F11}
    cls_off = {}
    off = 0
    for cn in CLS_NAMES:
        cls_off[cn] = off
        off += sizes[cn]
    S_TOT = off

    def pad_to(arr, size, fill):
        out = np.full(size, fill, dtype=np.int64)
        out[:arr.shape[0]] = arr
        return out

    maps = []
    for c in range(N_CORES):
        pu, pv, gp, ha, hb = per_core[c]
        cls = {}
        for hh, F in ((0, F00), (1, F11)):
            m = (ha == hh) & (hb == hh)
            fu, fv, fg = pu[m], pv[m], gp[m]
            nr = fu.shape[0]
            real = np.zeros(2 * F, dtype=bool)
            real[:nr] = True
            real[F:F + nr] = True
            cls[f"{hh}{hh}"] = (
                np.concatenate([pad_to(fu, F, hh * NH), pad_to(fv, F, hh * NH)]),
                np.concatenate([pad_to(fv, F, hh * NH), pad_to(fu, F, hh * NH)]),
                np.concatenate([pad_to(fg, F, -1), pad_to(fg + N_PAIRS, F, -1)]),
                real)
        m01 = (ha == 0) & (hb == 1)
        m10 = (ha == 1) & (hb == 0)
        au, av, ag = pu[m01], pv[m01], gp[m01]
        bu, bv, bg = pu[m10], pv[m10], gp[m10]
        nr = au.shape[0] + bu.shape[0]
        real = np.zeros(S01, dtype=bool)
        real[:nr] = True
        cls["01"] = (pad_to(np.concatenate([au, bv]), S01, 0),
                     pad_to(np.concatenate([av, bu]), S01, NH),
                     pad_to(np.concatenate([ag, bg + N_PAIRS]), S01, -1), real)
        cls["10"] = (pad_to(np.concatenate([av, bu]), S01, NH),
                     pad_to(np.concatenate([au, bv]), S01, 0),
                     pad_to(np.concatenate([ag + N_PAIRS, bg]), S01, -1),
                     real.copy())
        maps.append(cls)

    # seg layout: dense region (128 slots/window) + overflow (OV slots/window)
    OVR = _ceil_to(NWIN * OV, 128)
    SEG2 = NWIN * 128 + OVR

    meta = dict(F00=F00, F11=F11, S01=S01, sizes=sizes, cls_off=cls_off,
                S_TOT=S_TOT, SEG2=SEG2, OVR=OVR)

    bond_feats = np.asarray(bond_feats, dtype=np.float32)
    atom_pad = np.zeros((NPAD, ATOM_FDIM), dtype=np.float32)
    atom_pad[:N_NODES] = np.asarray(atom_feats, dtype=np.float32)
    atomT = np.ascontiguousarray(atom_pad.T).astype(F16)

    W_i = np.asarray(W_i, dtype=np.float32)
    W_h = np.asarray(W_h, dtype=np.float32)
    W_o = np.asarray(W_o, dtype=np.float32)
    wiaT = np.ascontiguousarray(W_i[:, :ATOM_FDIM].T).astype(F16)
    wibT = np.ascontiguousarray(W_i[:, ATOM_FDIM:].T).astype(F16)
    whT = np.ascontiguousarray(W_h.T).astype(F16)
    woaT = np.ascontiguousarray(W_o[:, :ATOM_FDIM].T).astype(F16)
    womT = np.ascontiguousarray(W_o[:, ATOM_FDIM:].T).astype(F16)

    iotaf = np.tile(np.arange(128, dtype=F16)[None, :], (128, 4)).copy()
    ident = np.eye(128, dtype=F16)

    shared = {
        "wia_a": wiaT[:128], "wia_b": wiaT[128:ATOM_FDIM], "wib": wibT,
        "wh": whT, "woa_a": woaT[:128], "woa_b": woaT[128:ATOM_FDIM],
        "wom": womT, "iotaf": iotaf, "ident": ident,
    }

    in_maps = []
    for c in range(N_CORES):
        cls = maps[c]
        src16_all = np.zeros(S_TOT, dtype=I16)
        bondT = np.zeros((BOND_FDIM, S_TOT), dtype=F16)
        seg16 = np.zeros(4 * SEG2, dtype=I16)
        # per class: NWIN dense one-hot cols then NWIN overflow cols
        dstl = np.full((128, 4 * 2 * NWIN), 200.0, dtype=F16)
        for ci, cn in enumerate(CLS_NAMES):
            s_half, d_half = int(cn[0]), int(cn[1])
            o, sz = cls_off[cn], sizes[cn]
            s_arr, d_arr, e_arr, real = cls[cn]
            src16_all[o:o + sz] = (s_arr - s_half * NH).astype(I16)
            realm = e_arr >= 0
            cols = np.zeros((BOND_FDIM, sz), dtype=F16)
            cols[:, realm] = bond_feats[e_arr[realm]].T.astype(F16)
            bondT[:, o:o + sz] = cols

            dl_all = d_arr - d_half * NH
            slots = np.nonzero(real)[0]
            dl = dl_all[real]
            w = dl // 128
            order = np.argsort(w, kind="stable")
            slots_s, dl_s, w_s = slots[order], dl[order], w[order]
            cnt = np.bincount(w_s, minlength=NWIN)
            starts = np.zeros(NWIN + 1, dtype=np.int64)
            np.cumsum(cnt, out=starts[1:])
            out_idx = np.zeros(SEG2, dtype=I16)
            dl_dense = np.full(NWIN * 128, 200.0, dtype=np.float32)
            dl_ov = np.full((128, NWIN), 200.0, dtype=np.float32)
            for wi in range(NWIN):
                a, b = starts[wi], starts[wi + 1]
                n = b - a
                nd = min(n, 128)
                base = wi * 128
                out_idx[base:base + nd] = slots_s[a:a + nd].astype(I16)
                dl_dense[base:base + nd] = (dl_s[a:a + nd] - wi * 128).astype(
                    np.float32)
                no = n - nd
                assert no <= OV, f"window overflow {no} > {OV}"
                if no:
                    ob = NWIN * 128 + wi * OV
                    out_idx[ob:ob + no] = slots_s[a + nd:b].astype(I16)
                    r0 = (wi % (128 // OV)) * OV
                    dl_ov[r0:r0 + no, wi] = (dl_s[a + nd:b] - wi * 128).astype(
                        np.float32)
            seg16[ci * SEG2:(ci + 1) * SEG2] = out_idx
            dstl[:, ci * 2 * NWIN:ci * 2 * NWIN + NWIN] = (
                dl_dense.reshape(NWIN, 128).T.astype(F16))
            dstl[:, ci * 2 * NWIN + NWIN:(ci + 1) * 2 * NWIN] = (
                dl_ov.astype(F16))

        # per-core atom columns: rows [c*NH8,(c+1)*NH8) of half 0 then half 1
        s0 = slice(c * NH8, (c + 1) * NH8)
        s1 = slice(NH + c * NH8, NH + (c + 1) * NH8)
        atomP_a = np.concatenate([atomT[:128, s0], atomT[:128, s1]], axis=1)
        atomP_b = np.concatenate([atomT[128:ATOM_FDIM, s0],
                                  atomT[128:ATOM_FDIM, s1]], axis=1)
        m = dict(shared)
        m["bondT"] = bondT
        m["src16w"] = _wrap_idx(src16_all)
        m["seg16w"] = _wrap_idx(seg16)
        m["dstl"] = dstl
        m["atomP_a"] = np.ascontiguousarray(atomP_a)
        m["atomP_b"] = np.ascontiguousarray(atomP_b)
        in_maps.append(m)

    return meta, in_maps


# ------------------------------------------------------------------ program
def _build_program(meta):
    import concourse.bacc as bacc
    import concourse.tile as tile
    import concourse.mybir as mybir
    from concourse import library_config

    NPAD, NWIN, OUT_COLS = _derived()
    f16, f32, i16 = mybir.dt.float16, mybir.dt.float32, mybir.dt.int16
    Relu = mybir.ActivationFunctionType.Relu

    sizes, cls_off = meta["sizes"], meta["cls_off"]
    S_TOT, SEG2, OVR = meta["S_TOT"], meta["SEG2"], meta["OVR"]
    F00, F11 = meta["F00"], meta["F11"]
    DENSE = NWIN * 128

    nc = bacc.Bacc("TRN2", target_bir_lowering=False, debug=False,
                   enable_asserts=False, num_devices=N_CORES,
                   num_swdge_queues=4)

    def din(name, shape, dt=f16):
        return nc.dram_tensor(name, shape, dt, kind="ExternalInput").ap()

    atomP_a = din("atomP_a", [128, OUT_COLS])
    atomP_b = din("atomP_b", [ATOM_FDIM - 128, OUT_COLS])
    wia_a = din("wia_a", [128, HIDDEN])
    wia_b = din("wia_b", [ATOM_FDIM - 128, HIDDEN])
    wib = din("wib", [BOND_FDIM, HIDDEN])
    wh_t = din("wh", [HIDDEN, HIDDEN])
    woa_a = din("woa_a", [128, HIDDEN])
    woa_b = din("woa_b", [ATOM_FDIM - 128, HIDDEN])
    wom = din("wom", [HIDDEN, HIDDEN])
    iotaf = din("iotaf", [128, 512])
    ident_t = din("ident", [128, 128])
    bondT = din("bondT", [BOND_FDIM, S_TOT])
    src16w = din("src16w", [128, S_TOT // 16], i16)
    seg16w = din("seg16w", [128, 4 * SEG2 // 16], i16)
    dstl_t = din("dstl", [128, 4 * 2 * NWIN])

    out_t = nc.dram_tensor("out", [HIDDEN, OUT_COLS], f32,
                           kind="ExternalOutput").ap()

    # proj: distributed compute + per-half AllGather
    myproj = nc.dram_tensor("myproj", [OUT_COLS, HIDDEN], f16,
                            kind="Internal").ap()
    proj_d = [nc.dram_tensor(f"proj{d}", [NH, HIDDEN], f16, kind="Internal",
                             addr_space="Shared").ap() for d in range(2)]
    h_cls = {}
    for ell in range(DEPTH):
        for cn in CLS_NAMES:
            h_cls[(ell, cn)] = nc.dram_tensor(
                f"h{ell}_{cn}", [sizes[cn], HIDDEN], f16, kind="Internal").ap()
    partials = [[nc.dram_tensor(f"partials{ell}_{d}", [NH, HIDDEN], f16,
                                kind="Internal").ap() for d in range(2)]
                for ell in range(DEPTH)]
    tmp = [[nc.dram_tensor(f"tmp{ell}_{d}", [NH, HIDDEN], f16,
                           kind="Internal", addr_space="Shared").ap()
            for d in range(2)] for ell in range(DEPTH - 1)]
    rs_out = [nc.dram_tensor(f"rsout{d}", [NH8, HIDDEN], f16,
                             kind="Internal").ap() for d in range(2)]

    nc.gpsimd.load_library(library_config.mlp)

    cls_of_d = {0: ["00", "10"], 1: ["01", "11"]}

    with tile.TileContext(nc) as tc:
        with (
            tc.tile_pool(name="pers", bufs=1) as pers,
            tc.tile_pool(name="work", bufs=2) as work,
            tc.tile_pool(name="segw", bufs=2) as segw,
            tc.tile_pool(name="psum", bufs=2, space="PSUM") as psum,
            tc.tile_pool(name="psum1", bufs=2, space="PSUM") as psum1,
            tc.tile_pool(name="psum2", bufs=4, space="PSUM") as psum2,
        ):
            # ---------- persistent SBUF
            def pload(ap_in, shape, tag, dt=f16, eng="sync"):
                t = pers.tile(shape, dt, tag=tag)
                (nc.sync if eng == "sync" else nc.gpsimd).dma_start(t[:], ap_in)
                return t

            w_wh = pload(wh_t[:], [HIDDEN, HIDDEN], "w_wh")
            w_wib = pload(wib[:], [BOND_FDIM, HIDDEN], "w_wib")
            w_wia_a = pload(wia_a[:], [128, HIDDEN], "w_wia_a")
            w_wia_b = pload(wia_b[:], [ATOM_FDIM - 128, HIDDEN], "w_wia_b")
            w_woa_a = pload(woa_a[:], [128, HIDDEN], "w_woa_a")
            w_woa_b = pload(woa_b[:], [ATOM_FDIM - 128, HIDDEN], "w_woa_b")
            w_wom = pload(wom[:], [HIDDEN, HIDDEN], "w_wom")
            io_t = pload(iotaf[:], [128, 512], "io_t")
            sidx = pload(src16w[:], [128, S_TOT // 16], "sidx", i16)
            gidx = pload(seg16w[:], [128, 4 * SEG2 // 16], "gidx", i16)
            dstl_s = pload(dstl_t[:], [128, 4 * 2 * NWIN], "dstl_s")
            ident_s = pload(ident_t[:], [128, 128], "ident_s")

            # ---------- distributed proj: rows [c*NH8,(c+1)*NH8) per half;
            # myproj rows [0,NH8) are this core's half-0 rows, [NH8,2*NH8)
            # its half-1 rows. AllGather each half slice when ready.
            pos = 0
            while pos < OUT_COLS:
                cw = min(BLK, OUT_COLS - pos)
                a_t = work.tile([128, BLK], f16, tag="pa")
                b_t = work.tile([ATOM_FDIM - 128, BLK], f16, tag="pb")
                nc.sync.dma_start(a_t[:, :cw], atomP_a[:, pos:pos + cw])
                nc.scalar.dma_start(b_t[:, :cw], atomP_b[:, pos:pos + cw])
                o_t = work.tile([128, BLK], f16, tag="po")
                for q in range(cw // 128):
                    ps = psum1.tile([128, 128], f32, tag="seg")
                    qs = slice(q * 128, (q + 1) * 128)
                    nc.tensor.matmul(ps[:], lhsT=a_t[:, qs],
                                     rhs=w_wia_a[:], start=True, stop=False)
                    nc.tensor.matmul(ps[:], lhsT=b_t[:, qs],
                                     rhs=w_wia_b[:], start=False, stop=True)
                    nc.scalar.copy(o_t[:, qs], ps[:])
                nc.sync.dma_start(
                    myproj[pos:pos + cw, :].rearrange(
                        "(a p) d -> p a d", p=128),
                    o_t[:, :cw].rearrange("p (a d) -> p a d", d=HIDDEN))
                pos += cw
            for d in range(2):
                nc.gpsimd.collective_compute(
                    "AllGather", mybir.AluOpType.bypass,
                    replica_groups=[list(range(N_CORES))],
                    ins=[myproj[d * NH8:(d + 1) * NH8, :]],
                    outs=[proj_d[d][:]])

            # ---------- layers
            for ell in range(DEPTH):
                for cn in CLS_NAMES:
                    s_half = int(cn[0])
                    o, sz = cls_off[cn], sizes[cn]
                    if ell == 0:
                        table = proj_d[s_half]
                    else:
                        table = tmp[ell - 1][s_half]
                    pos = 0
                    while pos < sz:
                        g = min(GOP, sz - pos)
                        nblk = g // BLK
                        icols = sidx[:, (o + pos) // 16:(o + pos + g) // 16]
                        h_t = work.tile([128, (GOP // 128) * HIDDEN], f16,
                                        tag="ht")
                        if ell == 0:
                            g1 = work.tile([128, (GOP // 128) * HIDDEN], f16,
                                           tag="g1", bufs=3)
                            nc.gpsimd.dma_gather(
                                g1[:, :(g // 128) * HIDDEN].rearrange(
                                    "p (c d) -> p c d", d=HIDDEN),
                                table[:], icols, g, g, HIDDEN,
                                single_packet=False)
                            bt = work.tile([BOND_FDIM, GOP], f16, tag="bt")
                            nc.scalar.dma_start(bt[:, :g],
                                                bondT[:, o + pos:o + pos + g])
                            for j in range(nblk):
                                ps = psum.tile([128, BLK], f32, tag="mm")
                                for q in range(4):
                                    ci = j * 4 + q
                                    nc.tensor.matmul(
                                        ps[:, q * 128:(q + 1) * 128],
                                        lhsT=bt[:, ci * 128:(ci + 1) * 128],
                                        rhs=w_wib[:], start=True, stop=True)
                                sl = slice(j * BLK, (j + 1) * BLK)
                                nc.vector.tensor_add(out=h_t[:, sl],
                                                     in0=g1[:, sl], in1=ps[:])
                                nc.scalar.activation(h_t[:, sl], h_t[:, sl],
                                                     Relu)
                        else:
                            # edge-major gather of tmp + edge-major rev read
                            g1 = work.tile([128, (GOP // 128) * HIDDEN], f16,
                                           tag="g1", bufs=3)
                            nc.gpsimd.dma_gather(
                                g1[:, :(g // 128) * HIDDEN].rearrange(
                                    "p (c d) -> p c d", d=HIDDEN),
                                table[:], icols, g, g, HIDDEN,
                                single_packet=False)
                            g2 = work.tile([128, (GOP // 128) * HIDDEN], f16,
                                           tag="g2", bufs=3)
                            for b1 in range(g // 1024):
                                rcn, rrow = _rev_row(cn, pos + b1 * 1024,
                                                     F00, F11)
                                nc.scalar.dma_start(
                                    g2[:, b1 * 8 * HIDDEN:
                                       (b1 + 1) * 8 * HIDDEN].rearrange(
                                        "p (c d) -> p c d", d=HIDDEN),
                                    h_cls[(ell - 1, rcn)]
                                    [rrow:rrow + 1024, :].rearrange(
                                        "(c p) d -> p c d", p=128))
                            nc.vector.tensor_tensor(
                                out=g1[:, :(g // 128) * HIDDEN],
                                in0=g1[:, :(g // 128) * HIDDEN],
                                in1=g2[:, :(g // 128) * HIDDEN],
                                op=mybir.AluOpType.subtract)
                            for j in range(nblk):
                                ps = psum.tile([128, BLK], f32, tag="mm")
                                mt = work.tile([128, BLK], f16, tag="mt")
                                for q in range(4):
                                    ci = j * 4 + q
                                    tp = psum2.tile([128, 128], f16, tag="tp")
                                    nc.tensor.transpose(
                                        tp[:], g1[:, ci * 128:(ci + 1) * 128],
                                        ident_s[:])
                                    msl = slice(q * 128, (q + 1) * 128)
                                    nc.scalar.copy(mt[:, msl], tp[:])
                                    nc.tensor.matmul(
                                        ps[:, q * 128:(q + 1) * 128],
                                        lhsT=mt[:, msl],
                                        rhs=w_wh[:], start=True, stop=True)
                                nc.scalar.activation(
                                    h_t[:, j * BLK:(j + 1) * BLK], ps[:], Relu)
                        nc.sync.dma_start(
                            h_cls[(ell, cn)][pos:pos + g, :].rearrange(
                                "(c p) d -> p c d", p=128),
                            h_t[:, :(g // 128) * HIDDEN].rearrange(
                                "p (c d) -> p c d", d=HIDDEN))
                        pos += g

                # ---------- segment sum -> partials[ell], collective per half
                # Per class: dense stream (128 slots/window, GOP chunks) plus
                # one overflow gather (OV slots/window, 128-slot chunks
                # covering 8 windows each).
                for d in (0, 1):
                    cur = {}
                    ovt = {}
                    s4 = {}
                    s4o = {}
                    for ii, cn in enumerate(cls_of_d[d]):
                        cur[cn] = dict(tile=None, base=-1, tag=f"sg{ii}")
                        ovt[cn] = dict(tile=None, tag=f"ov{ii}")
                        s4[cn] = dict(tile=None, base=-1, tag=f"oh{ii}")
                        s4o[cn] = dict(tile=None, base=-1, tag=f"oho{ii}")
                    for w in range(NWIN):
                        ps = psum1.tile([128, HIDDEN], f32, tag="seg")
                        k = 0
                        for cn in cls_of_d[d]:
                            ci = CLS_NAMES.index(cn)
                            slot = w * 128
                            gb = (slot // GOP) * GOP
                            if cur[cn]["base"] != gb:
                                g = min(GOP, DENSE - gb)
                                t = segw.tile(
                                    [128, (GOP // 128) * HIDDEN], f16,
                                    tag=cur[cn]["tag"])
                                nc.gpsimd.dma_gather(
                                    t[:, :(g // 128) * HIDDEN].rearrange(
                                        "p (c d) -> p c d", d=HIDDEN),
                                    h_cls[(ell, cn)][:],
                                    gidx[:, (ci * SEG2 + gb) // 16:
                                         (ci * SEG2 + gb + g) // 16],
                                    g, g, HIDDEN, single_packet=False)
                                cur[cn] = dict(tile=t, base=gb,
                                               tag=cur[cn]["tag"])
                            if ovt[cn]["tile"] is None:
                                t = segw.tile(
                                    [128, (OVR // 128) * HIDDEN], f16,
                                    tag=ovt[cn]["tag"], bufs=1)
                                nc.gpsimd.dma_gather(
                                    t[:].rearrange(
                                        "p (c d) -> p c d", d=HIDDEN),
                                    h_cls[(ell, cn)][:],
                                    gidx[:, (ci * SEG2 + DENSE) // 16:
                                         (ci * SEG2 + SEG2) // 16],
                                    OVR, OVR, HIDDEN, single_packet=False)
                                ovt[cn] = dict(tile=t, tag=ovt[cn]["tag"])
                            sb = (w // 4) * 4
                            if s4[cn]["base"] != sb:
                                st = segw.tile([128, 512], f16,
                                               tag=s4[cn]["tag"])
                                n4 = min(4, NWIN - sb)
                                dcol = ci * 2 * NWIN + sb
                                nc.vector.tensor_tensor(
                                    out=st[:, :n4 * 128].rearrange(
                                        "p (c n) -> p c n", n=128),
                                    in0=io_t[:, :n4 * 128].rearrange(
                                        "p (c n) -> p c n", n=128),
                                    in1=dstl_s[:, dcol:dcol + n4]
                                    .to_broadcast([128, n4, 128]),
                                    op=mybir.AluOpType.is_equal)
                                s4[cn] = dict(tile=st, base=sb,
                                              tag=s4[cn]["tag"])
                            if s4o[cn]["base"] != sb:
                                st = segw.tile([128, 512], f16,
                                               tag=s4o[cn]["tag"])
                                n4 = min(4, NWIN - sb)
                                dcol = ci * 2 * NWIN + NWIN + sb
                                nc.vector.tensor_tensor(
                                    out=st[:, :n4 * 128].rearrange(
                                        "p (c n) -> p c n", n=128),
                                    in0=io_t[:, :n4 * 128].rearrange(
                                        "p (c n) -> p c n", n=128),
                                    in1=dstl_s[:, dcol:dcol + n4]
                                    .to_broadcast([128, n4, 128]),
                                    op=mybir.AluOpType.is_equal)
                                s4o[cn] = dict(tile=st, base=sb,
                                               tag=s4o[cn]["tag"])
                            cb = (slot - cur[cn]["base"]) // 128
                            sq = w - s4[cn]["base"]
                            nc.tensor.matmul(
                                ps[:],
                                lhsT=s4[cn]["tile"][:, sq * 128:
                                                    (sq + 1) * 128],
                                rhs=cur[cn]["tile"][:, cb * HIDDEN:
                                                    (cb + 1) * HIDDEN],
                                start=(k == 0), stop=False)
                            k += 1
                            oc = w // (128 // OV)
                            nc.tensor.matmul(
                                ps[:],
                                lhsT=s4o[cn]["tile"][:, sq * 128:
                                                     (sq + 1) * 128],
                                rhs=ovt[cn]["tile"][:, oc * HIDDEN:
                                                    (oc + 1) * HIDDEN],
                                start=False, stop=(k == 3))
                            k += 1
                        p_t = segw.tile([128, HIDDEN], f16, tag="pt")
                        nc.scalar.copy(p_t[:], ps[:])
                        row = w * 128
                        nc.sync.dma_start(
                            partials[ell][d][row:row + 128, :].rearrange(
                                "(a p) d -> p a d", p=128),
                            p_t[:].unsqueeze(1))
                    # fire this half's collective as soon as its seg is done
                    if ell < DEPTH - 1:
                        nc.gpsimd.collective_compute(
                            "AllReduce", mybir.AluOpType.add,
                            replica_groups=[list(range(N_CORES))],
                            ins=[partials[ell][d][:]], outs=[tmp[ell][d][:]])
                    else:
                        nc.gpsimd.collective_compute(
                            "ReduceScatter", mybir.AluOpType.add,
                            replica_groups=[list(range(N_CORES))],
                            ins=[partials[ell][d][:]], outs=[rs_out[d][:]])

            # ---------- final: out.T = relu(WoA@atom.T + WoM@msg.T)
            # OUT_COLS columns; col j < NH8 -> rs_out[0][j], else rs_out[1]
            for ch in range(OUT_COLS // 128):
                csl = slice(ch * 128, (ch + 1) * 128)
                a_t = work.tile([128, 128], f16, tag="fa")
                b_t = work.tile([ATOM_FDIM - 128, 128], f16, tag="fb")
                m_t = work.tile([128, 128], f16, tag="fm")
                mraw = work.tile([128, 128], f16, tag="fmr")
                nc.sync.dma_start(a_t[:], atomP_a[:, csl])
                nc.scalar.dma_start(b_t[:], atomP_b[:, csl])
                r0 = ch * 128
                r1 = (ch + 1) * 128
                if r1 <= NH8:
                    nc.sync.dma_start(mraw[:], rs_out[0][r0:r1, :])
                elif r0 >= NH8:
                    nc.sync.dma_start(mraw[:], rs_out[1][r0 - NH8:r1 - NH8, :])
                else:
                    k = NH8 - r0
                    nc.sync.dma_start(mraw[:k, :], rs_out[0][r0:NH8, :])
                    nc.sync.dma_start(mraw[k:, :], rs_out[1][0:r1 - NH8, :])
                tpf = psum2.tile([128, 128], f16, tag="tp")
                nc.tensor.transpose(tpf[:], mraw[:], ident_s[:])
                nc.scalar.copy(m_t[:], tpf[:])
                ps = psum1.tile([128, 128], f32, tag="seg")
                nc.tensor.matmul(ps[:], lhsT=w_woa_a[:], rhs=a_t[:],
                                 start=True, stop=False)
                nc.tensor.matmul(ps[:], lhsT=w_woa_b[:], rhs=b_t[:],
                                 start=False, stop=False)
                nc.tensor.matmul(ps[:], lhsT=w_wom[:], rhs=m_t[:],
                                 start=False, stop=True)
                o_t = work.tile([128, 128], f32, tag="fo")
                nc.scalar.activation(o_t[:], ps[:], Relu)
                nc.sync.dma_start(out_t[:, csl], o_t[:])

    # Tile assigns SWDGE completion sems round-robin (DMASW<i>_*); the HW
    # locks each sem to one SWDGE queue, so spread gathers across the 4
    # queues by their assigned sem index.
    import re
    for b in nc.main_func.blocks:
        for ins in b.instructions:
            if type(ins).__name__ == "InstDMAGatherAnt" and ins.sync_info:
                for upd in ins.sync_info.on_update:
                    mname = upd.ant_name or ""
                    m = re.match(r"DMASW(\d+)_", mname)
                    if m:
                        ins.queue_num = int(m.group(1)) % 4
                        break

    nc.compile()
    return nc


def _rev_row(cn, slot, F00, F11):
    if cn == "01":
        return "10", slot
    if cn == "10":
        return "01", slot
    F = F00 if cn == "00" else F11
    return cn, (slot + F) if slot < F else (slot - F)


# -------------------------------------------------------------------- entry
_CACHE = {}


def kernel(atom_feats, bond_feats, W_i, W_h, W_o, src, dst, reverse_e):
    from concourse import bass_utils

    NPAD, NWIN, OUT_COLS = _derived()

    rev = np.asarray(reverse_e).astype(np.int64)
    ar = np.arange(N_PAIRS, dtype=np.int64)
    assert np.array_equal(rev[:N_PAIRS], ar + N_PAIRS) and \
        np.array_equal(rev[N_PAIRS:], ar), "unexpected reverse_e structure"

    meta, in_maps = _host_prep(atom_feats, bond_feats, W_i, W_h, W_o, src, dst)

    key = (meta["S_TOT"], meta["SEG2"], meta["F00"], meta["F11"],
           meta["S01"])
    if key not in _CACHE:
        _CACHE[key] = _build_program(meta)
    nc = _CACHE[key]

    res = bass_utils.run_bass_kernel_spmd(
        nc, in_maps, core_ids=list(range(N_CORES)))
    out = np.empty((NPAD, HIDDEN), dtype=np.float32)
    for c in range(N_CORES):
        o = res.results[c]["out"].T.astype(np.float32)   # [OUT_COLS, H]
        out[c * NH8:(c + 1) * NH8] = o[:NH8]
        out[NH + c * NH8:NH + (c + 1) * NH8] = o[NH8:]
    return np.ascontiguousarray(out[:N_NODES])

